# revision 1
# baseline (speedup 1.0000x reference)
"""GATv2 block (2 layers) on 8 Trainium2 NeuronCores via Bass/Tile.

Sharding: nodes graph-aligned across 8 cores; edges partitioned by destination
core and bucketed into 128-node destination windows; segment softmax and
aggregation stay local per core. BN stats via Gram-matrix AllReduce; one
fp16 AllGather of x between layers.

Edge phase: fp16 gather tables [*, D+H] whose last H columns hold the
0.6*att-projection, so the linear part of the GATv2 score rides along the
feature matmuls for free (lrelu(z) = 0.6z + 0.4|z|). Per tile, ee/Xsrc/Xdst
are summed in PSUM by the PE (edge-attr matmul + two identity injections);
|z| goes through the scalar engine; softmax num/den accumulate via a one-hot
segment matmul.
"""
import sys
import math

sys.path.insert(0, '/opt/trn_rl_repo')

import numpy as np
import concourse.bass as bass
import concourse.tile as tile
from concourse import bacc, mybir
from concourse.bass_utils import run_bass_kernel_spmd

F32 = mybir.dt.float32
F16 = mybir.dt.float16
I32 = mybir.dt.int32
AF = mybir.ActivationFunctionType
ALU = mybir.AluOpType

P = 128
NCORE = 8
NEG = 0.2
EPS = 1e-5
ASHIFT = -4.0   # constant softmax shift: exp(alpha-4) keeps fp16 exp in range
DEBUG = False
REPS = 1


# ----------------------------------------------------------------- host prep
def host_prep(x, node_batch, edge_index, edge_attr, Wl, bl, Wr, br, We, att,
              bias, Wres, W1, b1, bn_gamma, bn_beta, W2, b2, ln_gamma, ln_beta):
    N, D = x.shape
    E = edge_index.shape[1]
    ED = edge_attr.shape[1]
    L = Wl.shape[0]
    HID = W1.shape[2]
    G = int(node_batch.max()) + 1
    H = att.shape[1]
    C = att.shape[2]
    DA = D + H
    gpc = (G + NCORE - 1) // NCORE

    nb = np.asarray(node_batch).astype(np.int64)
    src = np.asarray(edge_index[0]).astype(np.int64)
    dst = np.asarray(edge_index[1]).astype(np.int64)
    ea = np.asarray(edge_attr, dtype=np.float32)

    gb = np.searchsorted(nb, np.arange(G + 1))
    n0s = np.array([gb[min(c * gpc, G)] for c in range(NCORE + 1)], dtype=np.int64)
    Nl = n0s[1:] - n0s[:-1]
    N_pad = int(math.ceil(max(Nl.max(), 1) / 512.0) * 512)
    W = N_pad // P
    NCH = N_pad // 512
    NPT = NCORE * N_pad

    core_of = np.searchsorted(n0s, np.arange(N), side='right') - 1
    glob_id = (core_of * N_pad + (np.arange(N) - n0s[core_of])).astype(np.int32)

    ecore = core_of[dst]
    counts = np.zeros((NCORE, W), dtype=np.int64)
    dslot_all = dst - n0s[ecore]
    ewin_all = dslot_all // P
    for c in range(NCORE):
        m = ecore == c
        counts[c] = np.bincount(ewin_all[m], minlength=W)
    T_w = np.maximum(np.ceil(counts.max(axis=0) / P).astype(np.int64), 1)
    tstart = np.concatenate([[0], np.cumsum(T_w)])
    nT = int(tstart[-1])
    E_pad = nT * P

    per_core = []
    for c in range(NCORE):
        m = ecore == c
        es = glob_id[src[m]]
        dslot = dslot_all[m]
        ew = dslot // P
        eat = ea[m]
        order = np.argsort(ew, kind='stable')
        es, dslot, ew, eat = es[order], dslot[order], ew[order], eat[order]
        srcf = np.zeros(E_pad, dtype=np.int32)
        dstf = np.zeros(E_pad, dtype=np.int32)
        offf = np.full(E_pad, -1.0, dtype=np.float16)
        eaf = np.zeros((ED + 1, E_pad), dtype=np.float16)
        wb = np.concatenate([[0], np.cumsum(counts[c])])
        for w in range(W):
            s0, s1 = wb[w], wb[w + 1]
            d0 = tstart[w] * P
            n = s1 - s0
            srcf[d0:d0 + n] = es[s0:s1]
            dstf[d0:d0 + n] = dslot[s0:s1]
            offf[d0:d0 + n] = (dslot[s0:s1] % P).astype(np.float16)
            eaf[:ED, d0:d0 + n] = eat[s0:s1].T.astype(np.float16)
            eaf[ED, d0:d0 + n] = 1.0
        per_core.append((srcf, dstf, offf, eaf))

    shared = {
        'iota_row': np.tile(np.arange(P, dtype=np.float16), (P, 1)),
        'giota_rep': np.tile(np.arange(gpc, dtype=np.float32), (P, 1)),
        'giota_col': np.arange(gpc, dtype=np.float32).reshape(gpc, 1),
        'ident': np.eye(P, dtype=np.float16),
        'ident32': np.eye(P, dtype=np.float32),
        'ones_col': np.ones((P, 1), np.float32),
        'ones_row': np.ones((1, 512), np.float32),
        'ones16': np.ones((1, P), np.float16),
    }
    for l in range(L):
        Ap = np.zeros((D, H), np.float32)   # 0.6 * block-diag att
        for h in range(H):
            Ap[h * C:(h + 1) * C, h] = 0.6 * np.asarray(att[l][h], np.float32)
        Wl_ = np.asarray(Wl[l], np.float32)
        Wr_ = np.asarray(Wr[l], np.float32)
        We_ = np.asarray(We[l], np.float32)
        bl_ = np.asarray(bl[l], np.float32)
        br_ = np.asarray(br[l], np.float32)
        shared[f'WlA{l}'] = np.concatenate([Wl_, Wl_ @ Ap], 1).astype(np.float16)
        shared[f'blA{l}'] = np.concatenate([bl_, bl_ @ Ap]).reshape(1, DA).astype(np.float16)
        shared[f'WrA{l}'] = np.concatenate([Wr_, Wr_ @ Ap], 1).astype(np.float16)
        wex = np.concatenate([We_, br_.reshape(1, D)], 0)
        shared[f'WeX{l}'] = np.concatenate([wex, wex @ Ap], 1).astype(np.float16)
        shared[f'Wres{l}'] = np.asarray(Wres[l], np.float16)
        shared[f'combo{l}'] = np.asarray(bias[l], np.float16).reshape(1, D)
        shared[f'att04{l}'] = np.tile(
            (0.4 * np.asarray(att[l], np.float32).reshape(1, H * C)).astype(np.float16), (P, 1))
        shared[f'W1_{l}'] = np.asarray(W1[l], np.float32)
        w2 = np.asarray(W2[l], np.float32)
        shared[f'W2_{l}'] = np.concatenate(
            [w2[k * P:(k + 1) * P, :] for k in range(HID // P)], axis=1)
        shared[f'b2_{l}'] = np.asarray(b2[l], np.float32).reshape(1, D)
        shared[f'bng{l}'] = np.asarray(bn_gamma[l], np.float32).reshape(1, HID)
        shared[f'bnb{l}'] = np.asarray(bn_beta[l], np.float32).reshape(1, HID)
        shared[f'lng{l}'] = np.asarray(ln_gamma[l], np.float32).reshape(D, 1)
        shared[f'lnb{l}'] = np.asarray(ln_beta[l], np.float32).reshape(D, 1)

    xT_glob = np.zeros((NCORE, P, N_pad), dtype=np.float16)
    xf = np.asarray(x, np.float32)
    for c in range(NCORE):
        xT_glob[c, :, :Nl[c]] = xf[n0s[c]:n0s[c + 1]].T.astype(np.float16)

    in_maps = []
    for c in range(NCORE):
        srcf, dstf, offf, eaf = per_core[c]
        lg = nb[n0s[c]:n0s[c + 1]] - c * gpc
        batch = np.full(N_pad, -1.0, np.float32)
        batch[:Nl[c]] = lg.astype(np.float32)
        valid = np.zeros(N_pad, np.float32)
        valid[:Nl[c]] = 1.0
        cnt = np.maximum(gb[np.minimum(c * gpc + np.arange(1, gpc + 1), G)]
                         - gb[np.minimum(c * gpc + np.arange(gpc), G)], 1)
        im = dict(shared)
        im.update({
            'srcidx': srcf.reshape(nT, P).T.copy(),
            'dstidx': dstf.reshape(nT, P).T.copy(),
            'dstoff_c': offf.reshape(nT, P).T.copy(),
            'eaT': eaf,
            'batch_row': batch.reshape(1, N_pad),
            'batch_col': batch.reshape(W, P).T.copy(),
            'valid_col': valid.reshape(W, P).T.copy(),
            'invcntD': (1.0 / (cnt * D)).astype(np.float32).reshape(gpc, 1),
            'xT_glob': xT_glob,
            'xT_loc': xT_glob[c].copy(),
        })
        in_maps.append(im)

    dims = dict(N=N, D=D, E=E, ED=ED, L=L, HID=HID, G=G, H=H, C=C, gpc=gpc,
                N_pad=N_pad, W=W, NCH=NCH, NPT=NPT, nT=nT, E_pad=E_pad,
                T_w=[int(t) for t in T_w], tstart=[int(t) for t in tstart],
                n0s=n0s, Nl=Nl)
    return in_maps, dims


# --------------------------------------------------------------- bass kernel
def build_nc(dims):
    D = dims['D']
    ED = dims['ED']
    L = dims['L']
    HID = dims['HID']
    H = dims['H']
    C = dims['C']
    DA = D + H
    gpc = dims['gpc']
    N_pad = dims['N_pad']
    W = dims['W']
    NCH = dims['NCH']
    NPT = dims['NPT']
    nT = dims['nT']
    E_pad = dims['E_pad']
    T_w = dims['T_w']
    tstart = dims['tstart']
    N = dims['N']
    HB = HID // P

    nc = bacc.Bacc("TRN2", target_bir_lowering=False, debug=False, num_devices=NCORE)

    def inp(name, shape, dt=F32):
        return nc.dram_tensor(name, list(shape), dt, kind="ExternalInput").ap()

    t_srcidx = inp('srcidx', (P, nT), I32)
    t_dstidx = inp('dstidx', (P, nT), I32)
    t_dstoff_c = inp('dstoff_c', (P, nT), F16)
    t_eaT = inp('eaT', (ED + 1, E_pad), F16)
    t_batch_row = inp('batch_row', (1, N_pad))
    t_batch_col = inp('batch_col', (P, W))
    t_valid_col = inp('valid_col', (P, W))
    t_invcntD = inp('invcntD', (gpc, 1))
    t_xT_glob = inp('xT_glob', (NCORE, P, N_pad), F16)
    t_xT_loc = inp('xT_loc', (P, N_pad), F16)
    t_iota_row = inp('iota_row', (P, P), F16)
    t_giota_rep = inp('giota_rep', (P, gpc))
    t_giota_col = inp('giota_col', (gpc, 1))
    t_ident = inp('ident', (P, P), F16)
    t_ident32 = inp('ident32', (P, P), F32)
    t_ones_col = inp('ones_col', (P, 1))
    t_ones_row = inp('ones_row', (1, 512))
    t_ones16 = inp('ones16', (1, P), F16)
    tw = {}
    wspec = []
    for l in range(L):
        wspec += [(f'WlA{l}', (P, DA), F16), (f'blA{l}', (1, DA), F16),
                  (f'WrA{l}', (P, DA), F16), (f'WeX{l}', (ED + 1, DA), F16),
                  (f'Wres{l}', (P, D), F16), (f'combo{l}', (1, D), F16),
                  (f'att04{l}', (P, H * C), F16),
                  (f'W1_{l}', (P, HID), F32), (f'W2_{l}', (P, HID), F32),
                  (f'b2_{l}', (1, D), F32), (f'bng{l}', (1, HID), F32),
                  (f'bnb{l}', (1, HID), F32),
                  (f'lng{l}', (D, 1), F32), (f'lnb{l}', (D, 1), F32)]
    for key, shape, dt in wspec:
        tw[key] = inp(key, shape, dt)

    t_out = nc.dram_tensor('out_rows', [N_pad, D], F32, kind="ExternalOutput").ap()
    t_dbg = {}
    if DEBUG:
        for l in range(L):
            t_dbg[f'x1T{l}'] = nc.dram_tensor(f'dbg_x1T{l}', [P, N_pad], F32, kind="ExternalOutput").ap()
            t_dbg[f'x2T{l}'] = nc.dram_tensor(f'dbg_x2T{l}', [P, N_pad], F32, kind="ExternalOutput").ap()

    with tile.TileContext(nc) as tc:
        with tc.tile_pool(name="const", bufs=1) as cpool, \
             tc.tile_pool(name="dram", bufs=1, space="DRAM") as dpool, \
             tc.tile_pool(name="big", bufs=1) as bigpool:

            def ld(ap, shape, dt=F32, pool=cpool, name=None):
                if name is None:
                    name = 'c_' + ap.tensor.name
                t = pool.tile(list(shape), dt, name=name, tag=name)
                nc.sync.dma_start(t[:], ap[:])
                return t

            s_srcidx = ld(t_srcidx, (P, nT), I32, bigpool)
            s_dstidx = ld(t_dstidx, (P, nT), I32, bigpool)
            s_dstoff_c = ld(t_dstoff_c, (P, nT), F16, bigpool)
            s_batch_col = ld(t_batch_col, (P, W))
            s_valid_col = ld(t_valid_col, (P, W))
            s_invcntD = ld(t_invcntD, (gpc, 1))
            s_iota_row = ld(t_iota_row, (P, P), F16)
            s_giota_rep = ld(t_giota_rep, (P, gpc))
            s_giota_col = ld(t_giota_col, (gpc, 1))
            s_ident = ld(t_ident, (P, P), F16)
            s_ident32 = ld(t_ident32, (P, P), F32)
            s_ones_col = ld(t_ones_col, (P, 1))
            s_ones_row = ld(t_ones_row, (1, 512))
            s_ones16 = ld(t_ones16, (1, P), F16)
            s_batch_row = ld(t_batch_row, (1, N_pad), F32, bigpool)
            sw = {}
            for key, shape, dt in wspec:
                sw[key] = ld(tw[key], shape, dt)

            d_xl = [dpool.tile([NPT, DA], F16, tag=f'xl{l}', name=f'd_xl{l}') for l in range(L)]
            d_xr = [dpool.tile([N_pad, DA], F16, tag=f'xr{l}', name=f'd_xr{l}') for l in range(L)]
            d_agin = dpool.tile([P, N_pad], F16, tag='agin')
            d_agout = dpool.tile([NCORE, P, N_pad], F16, tag='agout')
            d_x3loc = dpool.tile([P, N_pad], F16, tag='x3loc')
            d_arin = [dpool.tile([P, D + 1], F32, tag=f'arin{l}', name=f'd_arin{l}') for l in range(L)]
            d_arout = [dpool.tile([P, D + 1], F32, tag=f'arout{l}', name=f'd_arout{l}') for l in range(L)]

            x1T = bigpool.tile([P, N_pad], F32, tag='x1T')
            x2T = bigpool.tile([P, N_pad], F32, tag='x2T')

            for rep in range(REPS):
                for l in range(L):
                    xTg = t_xT_glob if l == 0 else d_agout
                    xTl = t_xT_loc if l == 0 else d_x3loc

                    # ======== Phase A: fp16 gather tables ========================
                    with tc.tile_pool(name="pA", bufs=6) as pA, \
                         tc.tile_pool(name="pAp", bufs=6, space="PSUM") as pAp:
                        for j in range(W):
                            lt = pA.tile([P, P], F16, tag='lhs')
                            nc.sync.dma_start(lt[:], xTl[:, j * P:(j + 1) * P])
                            ps = pAp.tile([P, DA], F32, tag='ps')
                            nc.tensor.matmul(ps[:], lhsT=lt[:], rhs=sw[f'WrA{l}'][:],
                                             start=True, stop=True)
                            ot = pA.tile([P, DA], F16, tag='o')
                            nc.vector.tensor_copy(ot[:], ps[:])
                            nc.sync.dma_start(d_xr[l][j * P:(j + 1) * P, :], ot[:])

                        for b in range(NCORE):
                            for j in range(W):
                                lt = pA.tile([P, P], F16, tag='lhs')
                                nc.sync.dma_start(lt[:], xTg[b, :, j * P:(j + 1) * P])
                                ps = pAp.tile([P, DA], F32, tag='ps')
                                nc.tensor.matmul(ps[:], lhsT=lt[:], rhs=sw[f'WlA{l}'][:],
                                                 start=True, stop=False)
                                nc.tensor.matmul(ps[:], lhsT=s_ones16[:, 0:1].to_broadcast([1, P]),
                                                 rhs=sw[f'blA{l}'][:], start=False, stop=True)
                                ot = pA.tile([P, DA], F16, tag='o')
                                nc.vector.tensor_copy(ot[:], ps[:])
                                r0 = b * N_pad + j * P
                                nc.sync.dma_start(d_xl[l][r0:r0 + P, :], ot[:])
                    # ======== Phase B: edge windows ==============================
                    with tc.tile_pool(name="pC", bufs=1) as pC:
                      with tc.tile_pool(name="pCs", bufs=1, space="PSUM") as pCsp:
                        pCs = pCsp.tile([P, D + 1], F32, tag='cs')
                        with tc.tile_pool(name="pB", bufs=2) as pB, \
                             tc.tile_pool(name="pB1", bufs=2) as pB1, \
                             tc.tile_pool(name="pBz", bufs=3, space="PSUM") as pBz, \
                             tc.tile_pool(name="pBa", bufs=2, space="PSUM") as pBa, \
                             tc.tile_pool(name="pBr", bufs=1, space="PSUM") as pBr:
                            for w in range(W):
                                T = T_w[w]
                                tb = tstart[w]
                                EW = T * P
                                eat = pB.tile([ED + 1, EW], F16, tag='eat')
                                nc.sync.dma_start(eat[:], t_eaT[:, tb * P:tb * P + EW])
                                xsrc = pB.tile([P, T * DA], F16, tag='xsrc')
                                xsv = xsrc[:].rearrange("p (t q) -> p t q", q=DA)
                                xdst = pB.tile([P, T * DA], F16, tag='xdst')
                                xdv = xdst[:].rearrange("p (t q) -> p t q", q=DA)
                                for t in range(T):
                                    nc.gpsimd.indirect_dma_start(
                                        out=xsv[:, t, :], out_offset=None,
                                        in_=d_xl[l][:],
                                        in_offset=bass.IndirectOffsetOnAxis(
                                            ap=s_srcidx[:, tb + t:tb + t + 1], axis=0))
                                    nc.gpsimd.indirect_dma_start(
                                        out=xdv[:, t, :], out_offset=None,
                                        in_=d_xr[l][:],
                                        in_offset=bass.IndirectOffsetOnAxis(
                                            ap=s_dstidx[:, tb + t:tb + t + 1], axis=0))
                                # S [e, n] one-hot (fp16)
                                S = pB1.tile([P, EW], F16, tag='S')
                                nc.vector.tensor_tensor(
                                    out=S[:].rearrange("p (t n) -> p t n", t=T),
                                    in0=s_iota_row[:].rearrange("p (o n) -> p o n", o=1).to_broadcast([P, T, P]),
                                    in1=s_dstoff_c[:, tb:tb + T].rearrange("p (t o) -> p t o", o=1).to_broadcast([P, T, P]),
                                    op=ALU.is_equal)
                                abs16 = pB1.tile([P, EW], F16, tag='abs16')
                                lin = pB1.tile([P, T * H], F32, tag='lin')
                                for t in range(T):
                                    pz = pBz.tile([P, DA], F32, tag='pz')
                                    nc.tensor.matmul(pz[:], lhsT=eat[:, t * P:(t + 1) * P],
                                                     rhs=sw[f'WeX{l}'][:], start=True, stop=False)
                                    nc.tensor.matmul(pz[:], lhsT=s_ident[:], rhs=xsv[:, t, :],
                                                     start=False, stop=False)
                                    nc.tensor.matmul(pz[:], lhsT=s_ident[:], rhs=xdv[:, t, :],
                                                     start=False, stop=True)
                                    nc.scalar.activation(out=abs16[:, t * P:(t + 1) * P],
                                                         in_=pz[:, 0:D], func=AF.Abs)
                                    nc.scalar.activation(out=lin[:, t * H:(t + 1) * H],
                                                         in_=pz[:, D:DA], func=AF.Copy,
                                                         bias=ASHIFT)
                                tabs = pB1.tile([P, EW], F16, tag='tabs')
                                nc.vector.tensor_tensor(
                                    out=tabs[:].rearrange("p (t n) -> p t n", t=T),
                                    in0=abs16[:].rearrange("p (t n) -> p t n", t=T),
                                    in1=sw[f'att04{l}'][:].rearrange("p (o n) -> p o n", o=1).to_broadcast([P, T, P]),
                                    op=ALU.mult)
                                alpha = pB.tile([P, T * H], F32, tag='alpha')
                                nc.vector.tensor_reduce(
                                    out=alpha[:],
                                    in_=tabs[:].rearrange("p (t h c) -> p t h c", h=H, c=C),
                                    axis=mybir.AxisListType.X, op=ALU.add)
                                alpha2 = pB.tile([P, T * H], F32, tag='alpha2')
                                nc.vector.tensor_add(alpha2[:], alpha[:], lin[:])
                                ybuf = pB.tile([P, T * (D + 8)], F16, tag='ybuf')
                                yv = ybuf[:].rearrange("p (t q) -> p t q", q=D + 8)
                                nc.scalar.activation(
                                    out=yv[:, :, D:D + 8],
                                    in_=alpha2[:].rearrange("p (t h) -> p t h", t=T),
                                    func=AF.Exp)
                                nc.vector.tensor_tensor(
                                    out=yv[:, :, 0:D].rearrange("p t (h c) -> p t h c", h=H),
                                    in0=xsv[:, :, 0:D].rearrange("p t (h c) -> p t h c", h=H),
                                    in1=yv[:, :, D:D + 8].rearrange("p t (h o) -> p t h o", o=1).to_broadcast([P, T, H, C]),
                                    op=ALU.mult)
                                pagg = pBa.tile([P, D + 8], F32, tag='pagg')
                                for t in range(T):
                                    nc.tensor.matmul(pagg[:], lhsT=S[:, t * P:(t + 1) * P],
                                                     rhs=yv[:, t, :], start=(t == 0),
                                                     stop=(t == T - 1))
                                pres = pBr.tile([P, D], F32, tag='pres')
                                lt = pB.tile([P, P], F16, tag='lres')
                                nc.sync.dma_start(lt[:], xTl[:, w * P:(w + 1) * P])
                                nc.tensor.matmul(pres[:], lhsT=lt[:], rhs=sw[f'Wres{l}'][:],
                                                 start=True, stop=False)
                                nc.tensor.matmul(pres[:], lhsT=s_ones16[:, 0:1].to_broadcast([1, P]),
                                                 rhs=sw[f'combo{l}'][:], start=False, stop=True)
                                den = pB.tile([P, H], F32, tag='den')
                                nc.vector.tensor_scalar(out=den[:], in0=pagg[:, D:D + 8],
                                                        scalar1=1e-16, scalar2=None, op0=ALU.add)
                                rec = pB.tile([P, H], F32, tag='rec')
                                nc.vector.reciprocal(rec[:], den[:])
                                x1w = pB.tile([P, D + 1], F32, tag='x1w')
                                nc.vector.tensor_tensor(
                                    out=x1w[:, 0:D].rearrange("p (h c) -> p h c", h=H),
                                    in0=pagg[:, 0:D].rearrange("p (h c) -> p h c", h=H),
                                    in1=rec[:].rearrange("p (h o) -> p h o", o=1).to_broadcast([P, H, C]),
                                    op=ALU.mult)
                                nc.vector.tensor_add(x1w[:, 0:D], x1w[:, 0:D], pres[:])
                                nc.vector.tensor_scalar(out=x1w[:, 0:D], in0=x1w[:, 0:D],
                                                        scalar1=s_valid_col[:, w:w + 1],
                                                        scalar2=None, op0=ALU.mult)
                                nc.vector.tensor_copy(x1w[:, D:D + 1], s_valid_col[:, w:w + 1])
                                nc.tensor.matmul(pCs[:], lhsT=x1w[:, 0:D], rhs=x1w[:, 0:D + 1],
                                                 start=(w == 0), stop=(w == W - 1),
                                                 skip_group_check=True)
                                ptr = pBr.tile([P, P], F32, tag='ptr')
                                nc.tensor.transpose(out=ptr[:], in_=x1w[:, 0:D], identity=s_ident32[:])
                                nc.vector.tensor_copy(x1T[:, w * P:(w + 1) * P], ptr[:])

                        if DEBUG:
                            nc.sync.dma_start(t_dbg[f'x1T{l}'][:], x1T[:])
                        # ======== Phase C: BN stats (AllReduce) ==================
                        pCp_cm = tc.tile_pool(name="pCp", bufs=1, space="PSUM")
                        pCp = pCp_cm.__enter__()
                        cs_sb = pC.tile([P, D + 1], F32, tag='cs')
                        nc.vector.tensor_copy(cs_sb[:], pCs[:])
                        nc.sync.dma_start(d_arin[l][:], cs_sb[:])
                        nc.gpsimd.collective_compute(
                            "AllReduce", ALU.add,
                            replica_groups=[list(range(NCORE))],
                            ins=[d_arin[l][:].opt()], outs=[d_arout[l][:].opt()])
                        csr = pC.tile([P, D + 1], F32, tag='csr')
                        nc.sync.dma_start(csr[:], d_arout[l][:])
                        mu = pC.tile([P, 1], F32, tag='mu')
                        nc.vector.tensor_scalar(out=mu[:], in0=csr[:, D:D + 1],
                                                scalar1=1.0 / N, scalar2=None, op0=ALU.mult)
                        pmu = pCp.tile([1, HID], F32, tag='pmu')
                        nc.tensor.matmul(pmu[:], lhsT=mu[:], rhs=sw[f'W1_{l}'][:],
                                         start=True, stop=True)
                        pP1 = pCp.tile([P, HID], F32, tag='pP1')
                        nc.tensor.matmul(pP1[:], lhsT=csr[:, 0:D], rhs=sw[f'W1_{l}'][:],
                                         start=True, stop=True)
                        w1p1 = pC.tile([P, HID], F32, tag='w1p1')
                        nc.vector.tensor_tensor(out=w1p1[:], in0=sw[f'W1_{l}'][:],
                                                in1=pP1[:], op=ALU.mult)
                        pt2 = pCp.tile([1, HID], F32, tag='pt2')
                        nc.tensor.matmul(pt2[:], lhsT=s_ones_col[:], rhs=w1p1[:],
                                         start=True, stop=True)
                        mh = pC.tile([1, HID], F32, tag='mh')
                        nc.vector.tensor_copy(mh[:], pmu[:])
                        var = pC.tile([1, HID], F32, tag='var')
                        nc.vector.tensor_scalar(out=var[:], in0=pt2[:], scalar1=1.0 / N,
                                                scalar2=None, op0=ALU.mult)
                        m2 = pC.tile([1, HID], F32, tag='m2')
                        nc.vector.tensor_tensor(out=m2[:], in0=mh[:], in1=mh[:], op=ALU.mult)
                        nc.vector.tensor_tensor(out=var[:], in0=var[:], in1=m2[:], op=ALU.subtract)
                        sd = pC.tile([1, HID], F32, tag='sd')
                        nc.vector.tensor_scalar(out=var[:], in0=var[:], scalar1=EPS,
                                                scalar2=None, op0=ALU.add)
                        nc.scalar.activation(out=sd[:], in_=var[:], func=AF.Sqrt)
                        rsd = pC.tile([1, HID], F32, tag='rsd')
                        nc.vector.reciprocal(rsd[:], sd[:])
                        geff = pC.tile([1, HID], F32, tag='geff')
                        nc.vector.tensor_tensor(out=geff[:], in0=sw[f'bng{l}'][:],
                                                in1=rsd[:], op=ALU.mult)
                        beff = pC.tile([1, HID], F32, tag='beff')
                        nc.vector.tensor_tensor(out=beff[:], in0=mh[:], in1=geff[:], op=ALU.mult)
                        nc.vector.tensor_tensor(out=beff[:], in0=sw[f'bnb{l}'][:],
                                                in1=beff[:], op=ALU.subtract)
                        pgrep = pCp.tile([P, HID], F32, tag='pgrep')
                        nc.tensor.matmul(pgrep[:], lhsT=s_ones_col[:1, :].rearrange("o p -> p o").to_broadcast([1, P]),
                                         rhs=geff[:], start=True, stop=True)
                        w1eff = pC.tile([P, HID], F32, tag='w1eff')
                        nc.vector.tensor_tensor(out=w1eff[:], in0=sw[f'W1_{l}'][:],
                                                in1=pgrep[:], op=ALU.mult)
                        becol = pC.tile([P, HB], F32, tag='becol')
                        for k in range(HB):
                            ptb = pCp.tile([P, 1], F32, tag='ptb')
                            nc.tensor.transpose(out=ptb[:], in_=beff[:, k * P:(k + 1) * P],
                                                identity=s_ident32[:1, :1])
                            nc.vector.tensor_copy(becol[:, k:k + 1], ptb[:])
                        pCp_cm.__exit__(None, None, None)

                        # ======== Phase D: MLP (sharded, T-space) ================
                        with tc.tile_pool(name="pD", bufs=2) as pD, \
                             tc.tile_pool(name="pDp", bufs=2, space="PSUM") as pDp, \
                             tc.tile_pool(name="pDx", bufs=2, space="PSUM") as pDx:
                            for i in range(NCH):
                                c0 = i * 512
                                px2 = pDx.tile([P, 512], F32, tag='px2')
                                for k in range(HB):
                                    ph = pDp.tile([P, 512], F32, tag='ph')
                                    nc.tensor.matmul(ph[:], lhsT=w1eff[:, k * P:(k + 1) * P],
                                                     rhs=x1T[:, c0:c0 + 512],
                                                     start=True, stop=True)
                                    hs = pD.tile([P, 512], F32, tag='hs')
                                    nc.scalar.activation(out=hs[:], in_=ph[:], func=AF.Relu,
                                                         bias=becol[:, k:k + 1], scale=1.0)
                                    nc.tensor.matmul(px2[:], lhsT=sw[f'W2_{l}'][:, k * P:(k + 1) * P],
                                                     rhs=hs[:], start=(k == 0), stop=False,
                                                     skip_group_check=True)
                                nc.tensor.matmul(px2[:], lhsT=sw[f'b2_{l}'][:],
                                                 rhs=s_ones_row[:], start=False, stop=True,
                                                 skip_group_check=True)
                                nc.vector.tensor_add(x2T[:, c0:c0 + 512], px2[:],
                                                     x1T[:, c0:c0 + 512])

                        if DEBUG:
                            nc.sync.dma_start(t_dbg[f'x2T{l}'][:], x2T[:])
                        # ======== Phase E: graph LayerNorm =======================
                        with tc.tile_pool(name="pE", bufs=2) as pE, \
                             tc.tile_pool(name="pEg", bufs=1, space="PSUM") as pEgp, \
                             tc.tile_pool(name="pEp", bufs=1, space="PSUM") as pEp:
                            pgs = pEgp.tile([gpc, 2], F32, tag='pgs')
                            for w in range(W):
                                sl = slice(w * P, (w + 1) * P)
                                sq = pE.tile([P, P], F32, tag='sq')
                                nc.vector.tensor_tensor(out=sq[:], in0=x2T[:, sl],
                                                        in1=x2T[:, sl], op=ALU.mult)
                                pcs = pEp.tile([1, 2 * P], F32, tag='pcs')
                                nc.tensor.matmul(pcs[:, 0:P], lhsT=s_ones_col[:], rhs=x2T[:, sl],
                                                 start=True, stop=True, skip_group_check=True)
                                nc.tensor.matmul(pcs[:, P:2 * P], lhsT=s_ones_col[:], rhs=sq[:],
                                                 start=True, stop=True, skip_group_check=True)
                                rows = pE.tile([1, 2 * P], F32, tag='rows')
                                nc.vector.tensor_copy(rows[:], pcs[:])
                                csc = pE.tile([P, 2], F32, tag='csc')
                                for q in range(2):
                                    ptb = pEp.tile([P, 1], F32, tag='ptb2')
                                    nc.tensor.transpose(out=ptb[:], in_=rows[:, q * P:(q + 1) * P],
                                                        identity=s_ident32[:1, :1])
                                    nc.vector.tensor_copy(csc[:, q:q + 1], ptb[:])
                                bg = pE.tile([P, gpc], F32, tag='bg')
                                nc.vector.tensor_scalar(out=bg[:], in0=s_giota_rep[:],
                                                        scalar1=s_batch_col[:, w:w + 1],
                                                        scalar2=None, op0=ALU.is_equal)
                                nc.tensor.matmul(pgs[:], lhsT=bg[:], rhs=csc[:],
                                                 start=(w == 0), stop=(w == W - 1),
                                                 skip_group_check=True)
                            gm = pE.tile([gpc, 1], F32, tag='gm')
                            nc.vector.tensor_tensor(out=gm[:], in0=pgs[:, 0:1],
                                                    in1=s_invcntD[:], op=ALU.mult)
                            e2 = pE.tile([gpc, 1], F32, tag='e2')
                            nc.vector.tensor_tensor(out=e2[:], in0=pgs[:, 1:2],
                                                    in1=s_invcntD[:], op=ALU.mult)
                            gv = pE.tile([gpc, 1], F32, tag='gv')
                            nc.vector.tensor_tensor(out=gv[:], in0=gm[:], in1=gm[:], op=ALU.mult)
                            nc.vector.tensor_tensor(out=gv[:], in0=e2[:], in1=gv[:], op=ALU.subtract)
                            sdg = pE.tile([gpc, 1], F32, tag='sdg')
                            nc.vector.tensor_scalar(out=gv[:], in0=gv[:], scalar1=EPS,
                                                    scalar2=None, op0=ALU.add)
                            nc.scalar.activation(out=sdg[:], in_=gv[:], func=AF.Sqrt)
                            ivg = pE.tile([gpc, 1], F32, tag='ivg')
                            nc.vector.reciprocal(ivg[:], sdg[:])
                            gmr = pE.tile([gpc, P], F32, tag='gmr')
                            nc.vector.tensor_copy(gmr[:], gm[:].to_broadcast([gpc, P]))
                            ivr = pE.tile([gpc, P], F32, tag='ivr')
                            nc.vector.tensor_copy(ivr[:], ivg[:].to_broadcast([gpc, P]))
                            for i in range(NCH):
                                c0 = i * 512
                                pbr = pEp.tile([gpc, 512], F32, tag='pbr')
                                nc.tensor.matmul(pbr[:],
                                                 lhsT=s_ones_col[:1, 0:1].to_broadcast([1, gpc]),
                                                 rhs=s_batch_row[:, c0:c0 + 512],
                                                 start=True, stop=True)
                                bgT = pE.tile([gpc, 512], F32, tag='bgT')
                                nc.vector.tensor_scalar(out=bgT[:], in0=pbr[:],
                                                        scalar1=s_giota_col[:],
                                                        scalar2=None, op0=ALU.is_equal)
                                pgm = pEp.tile([P, 512], F32, tag='pgm')
                                nc.tensor.matmul(pgm[:], lhsT=gmr[:], rhs=bgT[:],
                                                 start=True, stop=True)
                                piv = pEp.tile([P, 512], F32, tag='piv')
                                nc.tensor.matmul(piv[:], lhsT=ivr[:], rhs=bgT[:],
                                                 start=True, stop=True)
                                tmp = pE.tile([P, 512], F32, tag='tmp')
                                nc.vector.tensor_tensor(out=tmp[:], in0=x2T[:, c0:c0 + 512],
                                                        in1=pgm[:], op=ALU.subtract)
                                nc.vector.tensor_tensor(out=tmp[:], in0=tmp[:],
                                                        in1=piv[:], op=ALU.mult)
                                x3c = pE.tile([P, 512], F16, tag='x3c')
                                nc.vector.tensor_scalar(out=x3c[:], in0=tmp[:],
                                                        scalar1=sw[f'lng{l}'][:],
                                                        scalar2=sw[f'lnb{l}'][:],
                                                        op0=ALU.mult, op1=ALU.add)
                                if l == 0:
                                    nc.sync.dma_start(d_agin[:, c0:c0 + 512], x3c[:])
                                    nc.sync.dma_start(d_x3loc[:, c0:c0 + 512], x3c[:])
                                else:
                                    for q in range(4):
                                        ptb2 = pEp.tile([P, P], F16, tag='ptb2')
                                        nc.tensor.transpose(out=ptb2[:],
                                                            in_=x3c[:, q * P:(q + 1) * P],
                                                            identity=s_ident[:])
                                        orow = pE.tile([P, P], F32, tag='orow')
                                        nc.vector.tensor_copy(orow[:], ptb2[:])
                                        r0 = c0 + q * P
                                        nc.sync.dma_start(t_out[r0:r0 + P, :], orow[:])
                    if l == 0:
                        nc.gpsimd.collective_compute(
                            "AllGather", ALU.bypass,
                            replica_groups=[list(range(NCORE))],
                            ins=[d_agin[:].opt()], outs=[d_agout[:].opt()])

    nc.compile()
    return nc


# ---------------------------------------------------------------- entry point
_CACHE = {}


def kernel(**inputs):
    in_maps, dims = host_prep(**inputs)
    key = (DEBUG, REPS, dims['N'], dims['E'], dims['N_pad'], dims['nT'], tuple(dims['T_w']))
    if key not in _CACHE:
        _CACHE[key] = build_nc(dims)
    nc = _CACHE[key]
    res = run_bass_kernel_spmd(nc, in_maps, core_ids=list(range(NCORE)), trace=False)
    global _last_res, _last_dims
    _last_res, _last_dims = res, dims
    N, D = dims['N'], dims['D']
    out = np.zeros((N, D), dtype=np.float32)
    n0s, Nl = dims['n0s'], dims['Nl']
    for c in range(NCORE):
        out[n0s[c]:n0s[c + 1]] = res.results[c]['out_rows'][:Nl[c]]
    return out

